# revision 14
# baseline (speedup 1.0000x reference)
"""CompressedLinear on 8 Trainium2 NeuronCores.

out[b,s,o] = sum_i x[b,s,i] * (w_int8[o,i] * scale[o]) + bias[o]
  x: [4, 2048, 4096] f32, w_int8: [16384, 4096] int32 (codes in [-64,63]),
  scale/bias: [16384] f32 -> out: [4, 2048, 16384] f32

Strategy (tensor-parallel over out_features):
  - Each of the 8 cores owns a 2048-row slice of W/scale/bias and computes
    out[:, :, c*2048:(c+1)*2048]; x is replicated.
  - Weight codes are exact in bf16; scale is applied AFTER the matmul
    (per-out-feature), so the matmul itself is integer-exact in bf16.
  - x is rounded to a single bf16 array: the only error is x's bf16
    rounding (~0.17% rel L2), far under the 2e-2 gate, at 1x bf16 matmul
    cost.
  - A block of dummy matmuls on memset data runs during the load window
    so the PE HAM clock-gate is already at 8/8 when real work starts.
  - Per core loop: stationary operand = 128-token column block of x^T,
    moving operand = w^T; PSUM holds [128 tokens, 4x512 outfeat]; 32
    k-tiles x 4 banks = 128 matmuls per token tile, then a fused
    scale-mult + bias-add epilogue on DVE and a DMA store. The last tile
    runs bank-outer so each PSUM bank's epilogue+store overlaps the
    remaining banks' matmuls, shortening the serial tail.

All data layout transforms (transpose, hi/lo split, int8 narrowing,
scale/bias broadcast) are host-side numpy; gather is a concat.
"""

import os

import numpy as np
import ml_dtypes

BF16 = ml_dtypes.bfloat16

OUT, IN = 16384, 4096
B, S = 4, 2048
TOK = B * S            # 8192 tokens
NCORES = 8
OSH = OUT // NCORES    # 2048 out-features per core
KT = IN // 128         # 32 k-tiles
TT = TOK // 128        # 64 token tiles
NB = OSH // 512        # 4 psum banks per token tile
WCH = 2                # k-tiles per w chunk
NCH = KT // WCH        # 16 w chunks
NWARM = 40             # HAM warm-up matmuls

_last_results = None   # BassKernelResults of the most recent run (for test.py)


def _build_program():
    from contextlib import ExitStack

    import concourse.bass as bass
    import concourse.tile as tile
    from concourse import mybir

    f32 = mybir.dt.float32
    bf16 = mybir.dt.bfloat16

    nc = bass.Bass()
    xhi_d = nc.declare_dram_parameter("xhi", [TT, 128, KT, 128], bf16, isOutput=False)
    w_d = nc.declare_dram_parameter("w", [128, KT, OSH], bf16, isOutput=False)
    scale_d = nc.declare_dram_parameter("scale", [128, NB, 512], f32, isOutput=False)
    bias_d = nc.declare_dram_parameter("bias", [128, NB, 512], f32, isOutput=False)
    out_d = nc.declare_dram_parameter("out", [TT, 128, NB, 512], f32, isOutput=True)

    from concourse.tile import add_dep_helper

    with tile.TileContext(nc) as tc, ExitStack() as ctx:
        wpool = ctx.enter_context(tc.tile_pool(name="w", bufs=1))
        cpool = ctx.enter_context(tc.tile_pool(name="consts", bufs=1))
        xpool = ctx.enter_context(tc.tile_pool(name="x", bufs=2))
        opool = ctx.enter_context(tc.tile_pool(name="o", bufs=2))
        pspool = ctx.enter_context(tc.tile_pool(name="ps", bufs=2, space="PSUM"))

        # Hardware sync-wait slots are tiny (1 per PE LW/MM and per SWDGE
        # DMA, 2 per HWDGE DMA), and Tile's wait assignment is per-proc
        # minimal but not transitive. So every cross-engine dependency is
        # absorbed by a dedicated cheap "carrier" op on the consuming engine,
        # with explicit ordering edges so the scheduler keeps each carrier
        # ahead of its dependents and every instruction introduces at most
        # one new wait.
        def order(after, before):
            add_dep_helper(after.ins, before.ins, sync=False, reason="carrier order")

        # --- PE warm-up: dummy matmuls on memset data so the HAM clock
        # gate reaches 8/8 before the first real matmul. Results land in
        # the psum slot later recycled (and start=True-cleared) by t=1.
        warm_x = cpool.tile([128, 512], bf16, tag="warmx")
        nc.gpsimd.memset(warm_x[:], 0)
        warm_ps = pspool.tile([128, NB, 512], f32, tag="ps")
        for _ in range(NWARM):
            nc.tensor.matmul(
                warm_ps[:, 0, :], warm_x[:, :128], warm_x[:], start=True, stop=True
            )

        # w arrives in chunks (4 k-tiles each) so the first matmuls only
        # wait on the chunk they read, not the whole 16.8 MB load.
        w_sb = wpool.tile([128, KT, OSH], bf16)
        w_dmas = [
            nc.sync.dma_start(w_sb[:, k : k + 4, :], w_d[:, k : k + 4, :])
            for k in range(0, KT, 4)
        ]

        scale_sb = cpool.tile([128, NB, 512], f32, tag="scale")
        scale_dma = nc.sync.dma_start(scale_sb[:], scale_d[:])
        bias_sb = cpool.tile([128, NB, 512], f32, tag="bias")
        bias_dma = nc.sync.dma_start(bias_sb[:], bias_d[:])

        # Per-iteration disjoint scratch columns -> the carrier ops carry no
        # WAW deps of their own.
        scratch = cpool.tile([1, TT + NB], f32, tag="scratch")
        dummy = cpool.tile([1, 2 * TT], f32, tag="dummy")
        dveA = cpool.tile([1, TT], f32, tag="dveA")
        dveB = cpool.tile([1, TT], f32, tag="dveB")

        psum_readers = []  # the last psum reader per iteration
        last_mms = []  # final matmul per iteration
        out_dmas = []
        out_copies = []
        x_dmas = []
        adds = []
        swdge = []  # every SWDGE dma in issue order (lane-sem coverage)

        for t in range(TT):
            xhi = xpool.tile([128, KT, 128], bf16, tag="xhi")
            # POOL carrier chain, one wait each: gen-2 x-load DMA (its lane
            # sem would otherwise ride the new DMA as a WAW wait) and gen-2
            # matmuls (x slot readers), before the x-slot rewrite.
            ms1 = nc.gpsimd.memset(dummy[:, 2 * t : 2 * t + 1], 0)
            ms2 = nc.gpsimd.memset(dummy[:, 2 * t + 1 : 2 * t + 2], 0)
            order(ms2, ms1)
            if t >= 2:
                add_dep_helper(
                    ms1.ins, x_dmas[t - 2].ins, reason="x WAW lane via carrier"
                )
                add_dep_helper(
                    ms2.ins,
                    last_mms[t - 2].ins,
                    reason="x slot reuse gated on POOL carrier",
                )
            d1 = nc.gpsimd.dma_start(xhi[:], xhi_d[t])
            order(d1, ms2)
            x_dmas.append(d1)
            swdge.append(d1)

            # t=0 reuses the warm-up psum slot (start=True clears it).
            ps = warm_ps if t == 0 else pspool.tile([128, NB, 512], f32, tag="ps")
            # PE carrier: guard LDWEIGHTS absorbing the psum-slot-free (DVE)
            # wait so the first real matmul only waits on PE.
            guard = nc.tensor.ldweights(w_sb[:, 0, :128])
            if t >= 2:
                add_dep_helper(
                    guard.ins,
                    psum_readers[t - 2].ins,
                    reason="psum slot reuse gated on guard ldweights",
                )
            last = t == TT - 1
            first_mm = None
            # Last tile runs bank-outer so bank j's epilogue can start while
            # banks j+1.. are still accumulating.
            loop = (
                [(k, j) for j in range(NB) for k in range(KT)]
                if last
                else [(k, j) for k in range(KT) for j in range(NB)]
            )
            for k, j in loop:
                mm = nc.tensor.matmul(
                    ps[:, j, :],
                    xhi[:, k, :],
                    w_sb[:, k, j * 512 : (j + 1) * 512],
                    start=(k == 0),
                    stop=(k == KT - 1),
                )
                if first_mm is None:
                    first_mm = mm
            order(first_mm, guard)
            last_mms.append(mm)

            ob = opool.tile([128, NB, 512], f32)
            # DVE carriers: absorb the ob-slot WAR deps (gen-2 out-store DMA
            # and gen-2 POOL scratch copy) ahead of the scale-mult. At t=0
            # they also observe the scale/bias HWDGE sems.
            c1 = nc.vector.tensor_copy(dveA[:, t : t + 1], scale_sb[:1, 0, :1])
            c2 = nc.vector.tensor_copy(dveB[:, t : t + 1], bias_sb[:1, 0, :1])
            if t >= 2:
                add_dep_helper(
                    c1.ins, out_dmas[t - 2].ins, reason="ob reuse vs out dma"
                )
                add_dep_helper(
                    c2.ins, out_copies[t - 2].ins, reason="ob reuse vs pool copy"
                )
            if not last:
                mult = nc.vector.tensor_tensor(
                    ob[:], ps[:], scale_sb[:], mybir.AluOpType.mult
                )
                order(mult, c1)
                order(mult, c2)
                psum_readers.append(mult)
                add = nc.vector.tensor_tensor(
                    ob[:], ob[:], bias_sb[:], mybir.AluOpType.add
                )
                adds.append(add)
                # POOL carrier: RAW on ob -> absorbs the DVE wait ahead of
                # the out-store.
                cp = nc.gpsimd.tensor_copy(scratch[:, t : t + 1], ob[:1, 0, :1])
                od = nc.gpsimd.dma_start(out_d[t], ob[:])
                order(od, cp)
                out_copies.append(cp)
                out_dmas.append(od)
                swdge.append(od)
            else:
                # Per-bank epilogue + store, pipelined against the still-
                # running banks of this tile's matmuls.
                prev = None
                for j in range(NB):
                    mult = nc.vector.tensor_tensor(
                        ob[:, j, :], ps[:, j, :], scale_sb[:, j, :],
                        mybir.AluOpType.mult,
                    )
                    if j == 0:
                        order(mult, c1)
                        order(mult, c2)
                    add = nc.vector.tensor_tensor(
                        ob[:, j, :], ob[:, j, :], bias_sb[:, j, :],
                        mybir.AluOpType.add,
                    )
                    # POOL carrier per bank: absorbs the DVE wait so the
                    # store only carries its SWDGE lane sem.
                    cp = nc.gpsimd.tensor_copy(
                        scratch[:, TT + j : TT + j + 1], ob[:1, j, :1]
                    )
                    if prev is not None:
                        order(cp, prev)
                    od = nc.gpsimd.dma_start(out_d[t, :, j, :], ob[:, j, :])
                    order(od, cp)
                    out_copies.append(cp)
                    prev = od
                    adds.append(add)
                    out_dmas.append(od)
                    swdge.append(od)
                psum_readers.append(mult)

        # Tail carriers: SP nops, one wait each, observing every outstanding
        # sem (PE, DVE, ACT, Pool, all SWDGE lane sems, all HWDGE DMAs) so
        # the kernel-tail SP drain doesn't exceed its sync-wait slots.
        tail_deps = [
            last_mms[-1],
            adds[-1],
            out_copies[-1],
            scale_dma,
            bias_dma,
            *w_dmas,
        ]
        # SWDGE DMAs spread over 8 lane sems (assignment is not strictly
        # round-robin) -> observe a deep window of trailing DMAs.
        tail_deps += swdge[-20:]
        for i, dep in enumerate(tail_deps):
            nop = nc.engines[mybir.EngineType.SP].nop(
                nofuse=True, hint=f"tail_carrier_{i}"
            )
            add_dep_helper(nop.ins, dep.ins, reason="tail drain carrier")

    return nc


def kernel(x, weight_int8, scale, bias):
    global _last_results
    from concourse.bass_utils import run_bass_kernel_spmd

    x = np.asarray(x)
    weight_int8 = np.asarray(weight_int8)
    scale = np.asarray(scale, dtype=np.float32)
    bias = np.asarray(bias, dtype=np.float32)

    # x^T [IN, TOK] in bf16, tiled to [TT, 128p(IN), KT, 128(tok)]
    xT = np.ascontiguousarray(x.reshape(TOK, IN).astype(np.float32).T)
    x_hi = xT.astype(BF16)
    x_hi = np.ascontiguousarray(
        x_hi.reshape(KT, 128, TT, 128).transpose(2, 1, 0, 3)
    )

    in_maps = []
    for c in range(NCORES):
        wc = weight_int8[c * OSH : (c + 1) * OSH].astype(np.float32).astype(BF16)
        # w^T [IN, OSH] tiled to [128p(IN), KT, OSH]
        wp = np.ascontiguousarray(wc.T.reshape(KT, 128, OSH).transpose(1, 0, 2))
        sc = np.ascontiguousarray(
            np.broadcast_to(scale[c * OSH : (c + 1) * OSH], (128, OSH))
        ).reshape(128, NB, 512)
        bc = np.ascontiguousarray(
            np.broadcast_to(bias[c * OSH : (c + 1) * OSH], (128, OSH))
        ).reshape(128, NB, 512)
        in_maps.append({"xhi": x_hi, "w": wp, "scale": sc, "bias": bc})

    nc = _build_program()
    trace = bool(os.environ.get("KERNEL_TRACE"))
    kwargs = {}
    if trace:
        # Local-only profiling: stub the bucket upload and install the axon
        # NTFF hook (the image's antenv stub lacks axon_hooks).
        import sys
        import types

        from concourse import bass_utils as _bu

        _bu.upload_artifacts = lambda tmpdir: "local://" + tmpdir
        if "antenv.axon_hooks" not in sys.modules:
            import antenv

            mod = types.ModuleType("antenv.axon_hooks")
            _holder = [None]
            mod.set_axon_ntff_profile_hook = lambda h: _holder.__setitem__(0, h)
            mod.get_axon_ntff_profile_hook = lambda: _holder[0]
            antenv.axon_hooks = mod
            sys.modules["antenv.axon_hooks"] = mod
        from antenv.axon_hooks import (
            get_axon_ntff_profile_hook,
            set_axon_ntff_profile_hook,
        )

        if get_axon_ntff_profile_hook() is None:
            from trn_agent_boot.trn_boot import _ntff_profile_via_ctypes

            set_axon_ntff_profile_hook(
                _ntff_profile_via_ctypes(
                    os.environ.get("PJRT_LIBRARY_PATH", "/opt/axon/libaxon_pjrt.so")
                )
            )
        tmpdir = os.environ.get("KERNEL_TRACE_DIR")
        if tmpdir:
            os.makedirs(tmpdir, exist_ok=True)
            kwargs["tmpdir"] = tmpdir

    res = run_bass_kernel_spmd(
        nc,
        in_maps,
        list(range(NCORES)),
        trace=trace,
        **kwargs,
    )
    _last_results = res

    parts = [res.results[c]["out"].reshape(TOK, OSH) for c in range(NCORES)]
    return np.concatenate(parts, axis=1).reshape(B, S, OUT)


# revision 15
# speedup vs baseline: 1.0036x; 1.0036x over previous
"""CompressedLinear on 8 Trainium2 NeuronCores.

out[b,s,o] = sum_i x[b,s,i] * (w_int8[o,i] * scale[o]) + bias[o]
  x: [4, 2048, 4096] f32, w_int8: [16384, 4096] int32 (codes in [-64,63]),
  scale/bias: [16384] f32 -> out: [4, 2048, 16384] f32

Strategy (tensor-parallel over out_features):
  - Each of the 8 cores owns a 2048-row slice of W/scale/bias and computes
    out[:, :, c*2048:(c+1)*2048]; x is replicated.
  - Weight codes are exact in bf16; scale is applied AFTER the matmul
    (per-out-feature), so the matmul itself is integer-exact in bf16.
  - x is rounded to a single bf16 array: the only error is x's bf16
    rounding (~0.17% rel L2), far under the 2e-2 gate, at 1x bf16 matmul
    cost (an earlier hi/lo-split version paid 2x for precision nobody
    needed).
  - Per core loop: stationary operand = 128-token column block of x^T,
    moving operand = w^T; PSUM holds [128 tokens, 4x512 outfeat]; 32
    k-tiles x 4 banks = 128 matmuls per token tile, then a fused
    scale-mult + bias-add epilogue on DVE and a DMA store.
  - w is loaded in chunks so the first matmuls only wait on the chunk
    they read; the ramp-up is w-DMA-bound either way and the cold (HAM
    half-clock) matmuls hide inside that window.
  - The final token tile is split into two independent 2-bank PSUM tiles
    so the first half's epilogue+store overlaps the second half's
    matmuls, halving the serial tail after the last matmul.

All data layout transforms (transpose, int8->bf16 cast, scale/bias
broadcast) are host-side numpy; gather is a concat.
"""

import os

import numpy as np
import ml_dtypes

BF16 = ml_dtypes.bfloat16

OUT, IN = 16384, 4096
B, S = 4, 2048
TOK = B * S            # 8192 tokens
NCORES = 8
OSH = OUT // NCORES    # 2048 out-features per core
KT = IN // 128         # 32 k-tiles
TT = TOK // 128        # 64 token tiles
NB = OSH // 512        # 4 psum banks per token tile

_last_results = None   # BassKernelResults of the most recent run (for test.py)


def _build_program():
    from contextlib import ExitStack

    import concourse.bass as bass
    import concourse.tile as tile
    from concourse import mybir

    f32 = mybir.dt.float32
    bf16 = mybir.dt.bfloat16

    nc = bass.Bass()
    xhi_d = nc.declare_dram_parameter("xhi", [TT, 128, KT, 128], bf16, isOutput=False)
    w_d = nc.declare_dram_parameter("w", [128, KT, OSH], bf16, isOutput=False)
    scale_d = nc.declare_dram_parameter("scale", [128, NB, 512], f32, isOutput=False)
    bias_d = nc.declare_dram_parameter("bias", [128, NB, 512], f32, isOutput=False)
    out_d = nc.declare_dram_parameter("out", [TT, 128, NB, 512], f32, isOutput=True)

    from concourse.tile import add_dep_helper

    with tile.TileContext(nc) as tc, ExitStack() as ctx:
        wpool = ctx.enter_context(tc.tile_pool(name="w", bufs=1))
        cpool = ctx.enter_context(tc.tile_pool(name="consts", bufs=1))
        xpool = ctx.enter_context(tc.tile_pool(name="x", bufs=2))
        opool = ctx.enter_context(tc.tile_pool(name="o", bufs=2))
        pspool = ctx.enter_context(tc.tile_pool(name="ps", bufs=2, space="PSUM"))

        # Hardware sync-wait slots are tiny (1 per PE LW/MM and per SWDGE
        # DMA, 2 per HWDGE DMA), and Tile's wait assignment is per-proc
        # minimal but not transitive. So every cross-engine dependency is
        # absorbed by a dedicated cheap "carrier" op on the consuming engine,
        # with explicit ordering edges so the scheduler keeps each carrier
        # ahead of its dependents and every instruction introduces at most
        # one new wait.
        def order(after, before):
            add_dep_helper(after.ins, before.ins, sync=False, reason="carrier order")

        # w arrives in chunks (4 k-tiles each) so the first matmuls only
        # wait on the chunk they read, not the whole 16.8 MB load.
        w_sb = wpool.tile([128, KT, OSH], bf16)
        w_dmas = [
            nc.sync.dma_start(w_sb[:, k : k + 4, :], w_d[:, k : k + 4, :])
            for k in range(0, KT, 4)
        ]

        scale_sb = cpool.tile([128, NB, 512], f32, tag="scale")
        scale_dma = nc.sync.dma_start(scale_sb[:], scale_d[:])
        bias_sb = cpool.tile([128, NB, 512], f32, tag="bias")
        bias_dma = nc.sync.dma_start(bias_sb[:], bias_d[:])

        # Per-iteration disjoint scratch columns -> the carrier ops carry no
        # WAW deps of their own.
        scratch = cpool.tile([1, TT + 2], f32, tag="scratch")
        dummy = cpool.tile([1, 2 * TT], f32, tag="dummy")
        dveA = cpool.tile([1, TT + 2], f32, tag="dveA")
        dveB = cpool.tile([1, TT + 2], f32, tag="dveB")
        # Preamble DVE carriers: observe the scale/bias const loads on DVE so
        # no steady-state DVE op pairs a DMAHW wait with another wait.
        pre = cpool.tile([1, 2], f32, tag="pre")
        nc.vector.tensor_copy(pre[:, 0:1], scale_sb[:1, 0, :1])
        nc.vector.tensor_copy(pre[:, 1:2], bias_sb[:1, 0, :1])

        psum_readers = []  # the last psum reader per sub-iteration
        last_mms = []  # final matmul per sub-iteration
        out_dmas = []
        out_copies = []
        x_dmas = []
        adds = []
        swdge = []  # every SWDGE dma in issue order (lane-sem coverage)

        # Iteration plan: tiles 0..TT-2 process all NB banks at once; the
        # final tile is split into two 2-bank sub-iterations with separate
        # PSUM tiles, so the first half's epilogue overlaps the second
        # half's matmuls. Each entry: (t, j0, nb, new_x)
        plan = [(t, 0, NB, True) for t in range(TT - 1)]
        plan.append((TT - 1, 0, NB // 2, True))
        plan.append((TT - 1, NB // 2, NB - NB // 2, False))

        xhi = None
        for i, (t, j0, nb, new_x) in enumerate(plan):
            if new_x:
                xhi = xpool.tile([128, KT, 128], bf16, tag="xhi")
                # POOL carrier chain, one wait each: gen-2 x-load DMA (its
                # lane sem would otherwise ride the new DMA as a WAW wait)
                # and gen-2 matmuls (x slot readers), before the x-slot
                # rewrite.
                ms1 = nc.gpsimd.memset(dummy[:, 2 * t : 2 * t + 1], 0)
                ms2 = nc.gpsimd.memset(dummy[:, 2 * t + 1 : 2 * t + 2], 0)
                order(ms2, ms1)
                if t >= 2:
                    add_dep_helper(
                        ms1.ins, x_dmas[t - 2].ins, reason="x WAW lane via carrier"
                    )
                    add_dep_helper(
                        ms2.ins,
                        last_mms[t - 2].ins,
                        reason="x slot reuse gated on POOL carrier",
                    )
                d1 = nc.gpsimd.dma_start(xhi[:], xhi_d[t])
                order(d1, ms2)
                x_dmas.append(d1)
                swdge.append(d1)

            ps = pspool.tile([128, nb, 512], f32, tag="ps")
            # PE carrier: guard LDWEIGHTS absorbing the psum-slot-free (DVE)
            # wait so the first real matmul only waits on PE.
            guard = nc.tensor.ldweights(w_sb[:, 0, :128])
            if i >= 2:
                add_dep_helper(
                    guard.ins,
                    psum_readers[i - 2].ins,
                    reason="psum slot reuse gated on guard ldweights",
                )
            first_mm = None
            for k in range(KT):
                for j in range(nb):
                    mm = nc.tensor.matmul(
                        ps[:, j, :],
                        xhi[:, k, :],
                        w_sb[:, k, (j0 + j) * 512 : (j0 + j + 1) * 512],
                        start=(k == 0),
                        stop=(k == KT - 1),
                    )
                    if first_mm is None:
                        first_mm = mm
            order(first_mm, guard)
            last_mms.append(mm)

            ob = opool.tile([128, nb, 512], f32, tag="ob")
            # DVE carriers: absorb the ob-slot WAR deps (gen-2 out-store DMA
            # and gen-2 POOL scratch copy) ahead of the scale-mult.
            c1 = nc.vector.tensor_copy(dveA[:, i : i + 1], scale_sb[:1, 0, :1])
            c2 = nc.vector.tensor_copy(dveB[:, i : i + 1], scale_sb[:1, 0, :1])
            if i >= 2:
                add_dep_helper(
                    c1.ins, out_dmas[i - 2].ins, reason="ob reuse vs out dma"
                )
                add_dep_helper(
                    c2.ins, out_copies[i - 2].ins, reason="ob reuse vs pool copy"
                )
            mult = nc.vector.tensor_tensor(
                ob[:], ps[:], scale_sb[:, j0 : j0 + nb, :], mybir.AluOpType.mult
            )
            order(mult, c1)
            order(mult, c2)
            psum_readers.append(mult)
            adds.append(
                nc.vector.tensor_tensor(
                    ob[:], ob[:], bias_sb[:, j0 : j0 + nb, :], mybir.AluOpType.add
                )
            )
            # POOL carrier: RAW on ob -> absorbs the DVE wait ahead of the
            # out-store.
            cp = nc.gpsimd.tensor_copy(scratch[:, i : i + 1], ob[:1, 0, :1])
            od = nc.gpsimd.dma_start(out_d[t, :, j0 : j0 + nb, :], ob[:])
            order(od, cp)
            out_copies.append(cp)
            out_dmas.append(od)
            swdge.append(od)

        # Tail carriers: SP nops, one wait each, observing every outstanding
        # sem (PE, DVE, Pool, all SWDGE lane sems, all HWDGE DMAs) so the
        # kernel-tail SP drain doesn't exceed its sync-wait slots.
        tail_deps = [
            last_mms[-1],
            adds[-1],
            out_copies[-1],
            scale_dma,
            bias_dma,
            *w_dmas,
        ]
        # SWDGE DMAs spread over 8 lane sems (assignment is not strictly
        # round-robin) -> observe a deep window of trailing DMAs.
        tail_deps += swdge[-20:]
        for i, dep in enumerate(tail_deps):
            nop = nc.engines[mybir.EngineType.SP].nop(
                nofuse=True, hint=f"tail_carrier_{i}"
            )
            add_dep_helper(nop.ins, dep.ins, reason="tail drain carrier")

    return nc


def kernel(x, weight_int8, scale, bias):
    global _last_results
    from concourse.bass_utils import run_bass_kernel_spmd

    x = np.asarray(x)
    weight_int8 = np.asarray(weight_int8)
    scale = np.asarray(scale, dtype=np.float32)
    bias = np.asarray(bias, dtype=np.float32)

    # x^T [IN, TOK] in bf16, tiled to [TT, 128p(IN), KT, 128(tok)]
    xT = np.ascontiguousarray(x.reshape(TOK, IN).astype(np.float32).T)
    x_hi = xT.astype(BF16)
    x_hi = np.ascontiguousarray(
        x_hi.reshape(KT, 128, TT, 128).transpose(2, 1, 0, 3)
    )

    in_maps = []
    for c in range(NCORES):
        wc = weight_int8[c * OSH : (c + 1) * OSH].astype(np.float32).astype(BF16)
        # w^T [IN, OSH] tiled to [128p(IN), KT, OSH]
        wp = np.ascontiguousarray(wc.T.reshape(KT, 128, OSH).transpose(1, 0, 2))
        sc = np.ascontiguousarray(
            np.broadcast_to(scale[c * OSH : (c + 1) * OSH], (128, OSH))
        ).reshape(128, NB, 512)
        bc = np.ascontiguousarray(
            np.broadcast_to(bias[c * OSH : (c + 1) * OSH], (128, OSH))
        ).reshape(128, NB, 512)
        in_maps.append({"xhi": x_hi, "w": wp, "scale": sc, "bias": bc})

    nc = _build_program()
    trace = bool(os.environ.get("KERNEL_TRACE"))
    kwargs = {}
    if trace:
        # Local-only profiling: stub the bucket upload and install the axon
        # NTFF hook (the image's antenv stub lacks axon_hooks).
        import sys
        import types

        from concourse import bass_utils as _bu

        _bu.upload_artifacts = lambda tmpdir: "local://" + tmpdir
        if "antenv.axon_hooks" not in sys.modules:
            import antenv

            mod = types.ModuleType("antenv.axon_hooks")
            _holder = [None]
            mod.set_axon_ntff_profile_hook = lambda h: _holder.__setitem__(0, h)
            mod.get_axon_ntff_profile_hook = lambda: _holder[0]
            antenv.axon_hooks = mod
            sys.modules["antenv.axon_hooks"] = mod
        from antenv.axon_hooks import (
            get_axon_ntff_profile_hook,
            set_axon_ntff_profile_hook,
        )

        if get_axon_ntff_profile_hook() is None:
            from trn_agent_boot.trn_boot import _ntff_profile_via_ctypes

            set_axon_ntff_profile_hook(
                _ntff_profile_via_ctypes(
                    os.environ.get("PJRT_LIBRARY_PATH", "/opt/axon/libaxon_pjrt.so")
                )
            )
        tmpdir = os.environ.get("KERNEL_TRACE_DIR")
        if tmpdir:
            os.makedirs(tmpdir, exist_ok=True)
            kwargs["tmpdir"] = tmpdir

    res = run_bass_kernel_spmd(
        nc,
        in_maps,
        list(range(NCORES)),
        trace=trace,
        **kwargs,
    )
    _last_results = res

    parts = [res.results[c]["out"].reshape(TOK, OSH) for c in range(NCORES)]
    return np.concatenate(parts, axis=1).reshape(B, S, OUT)
